# revision 49
# baseline (speedup 1.0000x reference)
"""GAT (2-layer graph attention network) on 8 Trainium2 NeuronCores — v2.

Sharding (per spec hint): node dim N=4096 across 8 cores (512 rows each).
Each core computes its [512, 4096] attention slice per head in transposed
layout [j-partition, i-free]; h / s vectors are AllGathered.

Changes vs the 457us baseline:
- Host ships an additive mask bias mb = (adj-1)*1500 (bf16) instead of the
  raw 0/1 mask: exp(lrelu(t + mb)) == adj * exp(lrelu(t)) to f32 flush, so
  the per-tile mask multiply (a full DVE pass over 19M elems) disappears.
- A runtime-registered custom DVE op (LRELU_MSR_ANT) computes
  e = lrelu(mb + src + sdst) in ONE DVE instruction per j-block (the
  4-op stock chain needed ~1.9 cyc/elem; the fused op ~1.25 incl
  overhead, with f32 interior math). ~72% of groups take this route;
  the rest take an ACT route (TT add + per-jb Prelu + Exp) to balance
  the two engines. A-groups are emitted first per head so their TT runs
  while the s AllGather is in flight.
- Per-head h AllGathers issued as soon as each head's h is packed; the s
  AllGather emitted first (it gates all attends).
- The wall is dominated by DVE/ACT busy time under a core power throttle
  (util limit ~0.5-0.65 in NTFF), so minimizing per-element engine
  passes is what matters; PE-offload routes (identity-matmul combine)
  were tried and removed - PE matmul passes are slower per element than
  the engines they relieve.
"""
import sys
import time

sys.path.insert(0, "/opt/trn_rl_repo")

import numpy as np
import ml_dtypes

import concourse.bass as bass
import concourse.bacc as bacc
import concourse.tile as tile
from concourse import mybir
from concourse.bass_utils import run_bass_kernel_spmd
from concourse.masks import make_identity

import concourse.dve_ops as dvo
from concourse.dve_spec import Spec, Src0, Src1, C0, C1, maxx, lower, _has_src1
from concourse.dve_uop import DveOpSpec

dt = mybir.dt
BF = ml_dtypes.bfloat16


def _register_lrelu_msr():
    """Custom DVE op: out = lrelu(in0 + in1 + s0), slope s1.

    One DVE instruction per j-block replaces the 4-op chain
    (TS add-sdst, TT add-src, TS mult-alpha, TT max); interior math is
    f32 so it is also slightly more accurate than the chained bf16 form.
    """
    name = "LRELU_MSR_ANT"
    if name in dvo._SUB_OPCODE_FOR_NAME:
        return next(op for op in dvo.OPS if op.name == name)
    t = Src0 + Src1 + C0
    body = maxx(t, t * C1)

    def ref(in0, in1, s0, s1, imm2):
        x = in0.astype(np.float32) + in1.astype(np.float32) + s0
        return np.maximum(x, x * s1)

    spec = Spec(body=body, reference=ref)
    row = max(dvo._SUB_OPCODE_FOR_NAME.values()) + 1
    assert row < 0x20
    dvo._SUB_OPCODE_FOR_NAME[name] = row
    shas = {}
    for ver in ("v3", "v4"):
        s = DveOpSpec(name=name, opcode=row, uops=lower(spec, ver=ver),
                      rd1_en=_has_src1(spec))
        shas[ver] = s.sha(ver)
    op = dvo.DveOp(name, spec, subdim=False, uops_sha=shas)
    dvo.OPS.append(op)
    return op


LRELU_MSR = _register_lrelu_msr()

N, NFEAT, NHID, NHEAD, NCLASS = 4096, 1024, 64, 8, 32
NCORES = 8
R = N // NCORES          # 512 rows per core
NJB = N // 128           # 32 j-blocks
KCH = NFEAT // 128       # 8 full K chunks for x@W (+1 for bias row)
MASK_BIG = 1500.0
ALPHA = 0.2

# Route pattern per head: 8 groups of 4 j-blocks each.
# 'L' = fused custom-DVE lrelu route, 'A' = ACT-prelu route.
PATTERNS = [
    list("LALLALLL"),   # 6 L / 2 A
    list("LALLALAL"),   # 5 L / 3 A
]

_cached = {}


def _build_program():
    nc = bacc.Bacc("TRN2", target_bir_lowering=False, debug=False,
                   enable_asserts=False, num_devices=NCORES)

    xT = nc.dram_tensor("xT", [NFEAT + 1, R], dt.bfloat16, kind="ExternalInput").ap()
    wh = nc.dram_tensor("wh", [NHEAD, NFEAT + 1, NHID], dt.bfloat16, kind="ExternalInput").ap()
    mb = nc.dram_tensor("mb", [N, R], dt.bfloat16, kind="ExternalInput").ap()
    aT = nc.dram_tensor("aT", [NHEAD, NHID, 2], dt.bfloat16, kind="ExternalInput").ap()
    wo = nc.dram_tensor("wo", [NHEAD * NHID + 1, NCLASS], dt.bfloat16, kind="ExternalInput").ap()
    ao = nc.dram_tensor("ao", [NCLASS, 2], dt.float32, kind="ExternalInput").ap()
    out = nc.dram_tensor("out", [R, NCLASS], dt.float32, kind="ExternalOutput").ap()

    with tile.TileContext(nc, num_cores=NCORES) as tc:
        _emit(nc, tc, xT, wh, mb, aT, wo, ao, out)
    nc.compile()
    return nc


def _bcast_mid(ap2d, n):
    """[p, F] AP -> [p, n, F] AP with stride-0 middle dim."""
    return bass.AP(tensor=ap2d.tensor, offset=ap2d.offset,
                   ap=[ap2d.ap[0], [0, n], ap2d.ap[1]])


def _emit(nc, tc, xT, wh, mb, aT, wo, ao, out):
    from contextlib import ExitStack
    f32, bf16 = dt.float32, dt.bfloat16
    AF = mybir.ActivationFunctionType
    OP = mybir.AluOpType
    AG = "AllGather"

    cst_ctx = ExitStack()
    cst = cst_ctx.enter_context(tc.tile_pool(name="cst", bufs=1))
    dram = cst_ctx.enter_context(tc.tile_pool(name="dram", bufs=1, space="DRAM"))

    # ---- collective buffers ----
    cc_s_in = dram.tile([2 * NHEAD, R], f32)
    cc_s_out = dram.tile([NCORES, 2 * NHEAD, R], f32, addr_space="Shared")
    cc_h_in = [dram.tile([R, NHID], bf16, name=f"cc_h_in{h}") for h in range(NHEAD)]
    cc_h_out = [dram.tile([NCORES, R, NHID], bf16, addr_space="Shared",
                          name=f"cc_h_out{h}") for h in range(NHEAD)]
    cc_ho_in = dram.tile([R, NCLASS], bf16)
    cc_ho_out = dram.tile([NCORES, R, NCLASS], bf16, addr_space="Shared")
    cc_s2_in = dram.tile([2, R], f32)
    cc_s2_out = dram.tile([NCORES, 2, R], f32, addr_space="Shared")
    groups = [list(range(NCORES))]

    # ---- persistent SBUF ----
    mbT = cst.tile([128, NJB, R], bf16)           # additive mask bias (0 / -1500)

    h_rhs = [cst.tile([128, NJB, NHID + 1], bf16, name=f"h_rhs{h}")
             for h in range(NHEAD)]
    for h in range(NHEAD):
        nc.vector.memset(h_rhs[h][:, :, NHID:NHID + 1], 1.0)

    src_bc = [cst.tile([128, R], bf16, name=f"src_bc{h}") for h in range(NHEAD)]
    sdst = cst.tile([128, NHEAD, NJB], f32)
    ident64 = cst.tile([64, 64], bf16)
    make_identity(nc, ident64)
    ident128 = cst.tile([128, 128], f32)
    make_identity(nc, ident128)
    ident33 = cst.tile([NCLASS + 1, NCLASS + 1], f32)
    make_identity(nc, ident33)
    ones64 = cst.tile([1, 64], f32)
    nc.vector.memset(ones64, 1.0)
    ones128 = cst.tile([1, 128], f32)
    nc.vector.memset(ones128, 1.0)
    ones_row = cst.tile([1, R], bf16)
    nc.vector.memset(ones_row, 1.0)
    xcatT = [cst.tile([128, R], bf16, name=f"xcatT{k}") for k in range(4)]
    h2_rhs = cst.tile([128, NJB, NCLASS + 1], bf16)
    nc.vector.memset(h2_rhs[:, :, NCLASS:NCLASS + 1], 1.0)
    src2_bc = cst.tile([128, R], bf16)
    s2dst = cst.tile([128, NJB], f32)

    # =================== Stage A: h = x @ W per head, s vectors ============
    stA = ExitStack()
    sa = stA.enter_context(tc.tile_pool(name="sa", bufs=1))
    psA = stA.enter_context(tc.tile_pool(name="psA", bufs=1, space="PSUM"))

    xT_sb = sa.tile([128, KCH + 1, R], bf16)
    nc.sync.dma_start(out=xT_sb[:, 0:KCH, :],
                      in_=xT[0:NFEAT, :].rearrange("(k p) i -> p k i", p=128))
    nc.sync.dma_start(out=xT_sb[0:1, KCH, :], in_=xT[NFEAT:NFEAT + 1, :])
    wh_sb = sa.tile([128, NHEAD, KCH + 1, NHID], bf16)
    for h in range(NHEAD):
        nc.sync.dma_start(out=wh_sb[:, h, 0:KCH, :],
                          in_=wh[h, 0:NFEAT, :].rearrange("(k p) o -> p k o", p=128))
        nc.sync.dma_start(out=wh_sb[0:1, h, KCH, :], in_=wh[h, NFEAT:NFEAT + 1, :])
    aT_sb = sa.tile([64, NHEAD, 2], bf16)
    nc.sync.dma_start(out=aT_sb, in_=aT.rearrange("h o k -> o h k"))
    nc.sync.dma_start(out=mbT, in_=mb.rearrange("(jb p) i -> p jb i", p=128))

    hT_sb = sa.tile([64, NHEAD, R], bf16)
    s1_all = sa.tile([2, NHEAD, R], f32)
    # all 8 heads' s vectors FIRST (they gate the s AllGather, which gates
    # every attend tile); the src broadcasts can wait.
    for h in range(NHEAD):
        ps_hT = psA.tile([64, R], f32, tag="hT", bufs=2)
        for k in range(KCH + 1):
            kp = 128 if k < KCH else 1
            nc.tensor.matmul(ps_hT, lhsT=wh_sb[0:kp, h, k, :],
                             rhs=xT_sb[0:kp, k, :],
                             start=(k == 0), stop=(k == KCH))
        nc.scalar.copy(out=hT_sb[:, h, :], in_=ps_hT)
        ps_s1 = psA.tile([2, R], f32, tag="s1", bufs=2)
        nc.tensor.matmul(ps_s1, lhsT=aT_sb[:, h, :], rhs=hT_sb[:, h, :],
                         start=True, stop=True)
        nc.vector.tensor_copy(out=s1_all[:, h, :], in_=ps_s1)
        nc.sync.dma_start(out=cc_s_in[2 * h:2 * h + 2, :], in_=s1_all[:, h, :])
    for h in range(NHEAD):
        ps_src = psA.tile([128, R], f32, tag="srcbc", bufs=2)
        nc.tensor.matmul(ps_src, lhsT=ones128, rhs=s1_all[0:1, h, :],
                         start=True, stop=True)
        nc.vector.tensor_copy(out=src_bc[h], in_=ps_src)

    # s AllGather first: it gates every attend tile.
    nc.gpsimd.collective_compute(AG, mybir.AluOpType.bypass, replica_groups=groups,
                                 ins=[cc_s_in[:]], outs=[cc_s_out[:]])
    for h in range(NHEAD):
        for core in range(NCORES):
            nc.sync.dma_start(
                out=sdst[:, h, core * 4:(core + 1) * 4],
                in_=cc_s_out[core, 2 * h + 1, :].rearrange("(l p) -> p l", p=128))

    # pack + gather h per head, issued as soon as each head's h is ready
    for h in range(NHEAD):
        h_row4 = sa.tile([128, 4, 64], bf16, tag="hrow", bufs=2)
        for tb in range(4):
            ps_tr = psA.tile([128, 64], bf16, tag="tr", bufs=2)
            nc.tensor.transpose(ps_tr, hT_sb[:, h, tb * 128:(tb + 1) * 128], ident64)
            nc.vector.tensor_copy(out=h_row4[:, tb, :], in_=ps_tr)
        nc.sync.dma_start(out=cc_h_in[h].rearrange("(l p) o -> p l o", p=128),
                          in_=h_row4)
        nc.gpsimd.collective_compute(AG, mybir.AluOpType.bypass, replica_groups=groups,
                                     ins=[cc_h_in[h][:]], outs=[cc_h_out[h][:]])
        nc.sync.dma_start(
            out=h_rhs[h][:, :, 0:NHID].rearrange("p (c l) o -> p c l o", c=NCORES),
            in_=cc_h_out[h].rearrange("c (l p) o -> p c l o", p=128))

    stA.close()

    # =================== Stage B: layer-1 attention ========================
    stB = ExitStack()
    sb_ = stB.enter_context(tc.tile_pool(name="sb", bufs=1))

    # stage-C pools opened early: h_out accumulates as head pairs finish.
    # PSUM pools close LIFO, so psC (closed later) must open before psB.
    stC = ExitStack()
    sc = stC.enter_context(tc.tile_pool(name="sc", bufs=1))
    psC_ctx = ExitStack()
    psC = psC_ctx.enter_context(tc.tile_pool(name="psC", bufs=1, space="PSUM"))

    psB_ctx = ExitStack()
    psB = psB_ctx.enter_context(tc.tile_pool(name="psB", bufs=1, space="PSUM"))

    wo_sb = sc.tile([128, 5, NCLASS], bf16)
    nc.sync.dma_start(out=wo_sb[:, 0:4, :],
                      in_=wo[0:NHEAD * NHID, :].rearrange("(k p) c -> p k c", p=128))
    nc.sync.dma_start(out=wo_sb[0:1, 4, :], in_=wo[NHEAD * NHID:NHEAD * NHID + 1, :])
    ao_sb = sc.tile([32, 2], f32)
    nc.sync.dma_start(out=ao_sb, in_=ao)
    ps_ho = psC.tile([128, 4, NCLASS], f32)

    def emit_group(psum_pool, route, g, src_tile, sdst_ap_fn, sink, cnt):
        """Emit one 4-block group; sink(jb, q_slice) per block. Returns cnt."""
        jb0 = 4 * g if not isinstance(g, tuple) else None
        if route == 'L':
            # route 'L' handles one OR two consecutive groups (g may be a
            # tuple): pairing halves the per-instruction Exp overhead.
            gs = g if isinstance(g, tuple) else (g,)
            njb = 4 * len(gs)
            jbs = [4 * gg + j4 for gg in gs for j4 in range(4)]
            e = sb_.tile([128, njb, R], bf16, tag=f"e{len(gs)}", bufs=2)
            for i, jb in enumerate(jbs):
                nc.vector._custom_dve(LRELU_MSR, out=e[:, i, :],
                                      in0=mbT[:, jb, :], in1=src_tile,
                                      s0=sdst_ap_fn(jb), s1=ALPHA)
            q = sb_.tile([128, njb, R], bf16, tag=f"q{len(gs)}", bufs=3 if len(gs)==1 else 2)
            nc.scalar.activation(out=q, in_=e, func=AF.Exp)
            for i, jb in enumerate(jbs):
                sink(cnt, jb, q[:, i, :])
                cnt += 1
        else:  # 'A'
            t = sb_.tile([128, 4, R], bf16, tag="t", bufs=3)
            nc.vector.tensor_tensor(out=t, in0=mbT[:, jb0:jb0 + 4, :],
                                    in1=_bcast_mid(src_tile, 4), op=OP.add)
            e = sb_.tile([128, 4, R], bf16, tag="e", bufs=2)
            for j4 in range(4):
                jb = jb0 + j4
                nc.scalar.activation(out=e[:, j4, :], in_=t[:, j4, :],
                                     func=AF.Prelu, bias=sdst_ap_fn(jb),
                                     scale=1.0, alpha=ALPHA)
            q = sb_.tile([128, 4, R], bf16, tag="q", bufs=3)
            nc.scalar.activation(out=q, in_=e, func=AF.Exp)
            for j4 in range(4):
                sink(cnt, jb0 + j4, q[:, j4, :])
                cnt += 1
        return cnt

    for h in range(NHEAD):
        ps_att = psB.tile([NHID + 1, R], f32, tag="att", bufs=3)

        def sink(cnt, jb, qs, ps_att=ps_att, h=h):
            nc.tensor.matmul(ps_att, lhsT=h_rhs[h][:, jb, :], rhs=qs,
                             start=(cnt == 0), stop=(cnt == NJB - 1))

        pat = PATTERNS[h % 2]
        cnt = 0
        # A-groups first (their mask+src TT needs no gathered data); then
        # L-groups in pairs to halve the Exp instruction count.
        items = [('A', g) for g in range(8) if pat[g] == 'A']
        l_gs = [g for g in range(8) if pat[g] == 'L']
        while l_gs:
            if len(l_gs) >= 2:
                items.append(('L', (l_gs[0], l_gs[1]))); l_gs = l_gs[2:]
            else:
                items.append(('L', l_gs[0])); l_gs = l_gs[1:]
        for route, g in items:
            cnt = emit_group(psB, route, g, src_bc[h],
                             lambda jb, h=h: sdst[:, h, jb:jb + 1], sink, cnt)

        # normalize + ELU -> x_catT
        dinv = sb_.tile([1, R], f32, tag="dinv", bufs=2)
        nc.vector.reciprocal(out=dinv, in_=ps_att[NHID:NHID + 1, :])
        ps_bc = psB.tile([64, R], f32, tag="bc", bufs=2)
        nc.tensor.matmul(ps_bc, lhsT=ones64, rhs=dinv, start=True, stop=True)
        att_sb = sb_.tile([64, R], f32, tag="attsb", bufs=2)
        nc.scalar.copy(out=att_sb, in_=ps_att[0:NHID, :])
        nc.vector.tensor_tensor(out=att_sb, in0=att_sb, in1=ps_bc, op=OP.mult)
        neg = sb_.tile([64, R], f32, tag="neg", bufs=2)
        nc.vector.tensor_scalar(out=neg, in0=att_sb, scalar1=0.0, scalar2=None,
                                op0=OP.min)
        q2_ = sb_.tile([64, R], f32, tag="q2e", bufs=2)
        nc.scalar.activation(out=q2_, in_=neg, func=AF.Exp)
        pos = sb_.tile([64, R], f32, tag="pos", bufs=2)
        nc.vector.tensor_scalar(out=pos, in0=att_sb, scalar1=0.0, scalar2=-1.0,
                                op0=OP.max, op1=OP.add)
        nc.vector.tensor_tensor(out=xcatT[h // 2][64 * (h % 2):64 * (h % 2) + 64, :],
                                in0=pos, in1=q2_, op=OP.add)

    psB_ctx.close()

    # =================== Stage C: h_out = x_cat @ W_out ====================
    for ib in range(4):
        isl = slice(ib * 128, (ib + 1) * 128)
        for k in range(5):
            if k < 4:
                nc.tensor.matmul(ps_ho[:, ib, :], lhsT=xcatT[k][:, isl],
                                 rhs=wo_sb[:, k, :], start=(k == 0), stop=False)
            else:
                nc.tensor.matmul(ps_ho[:, ib, :], lhsT=ones_row[:, isl],
                                 rhs=wo_sb[0:1, 4, :], start=False, stop=True)
    h_out_sb = sc.tile([128, 4, NCLASS], f32)
    nc.scalar.copy(out=h_out_sb, in_=ps_ho)
    # s2 path FIRST: its AllGather gates the layer-2 L-route elementwise,
    # so its input DMA must be ready before the (bigger) h_out gather's.
    houtT = sc.tile([32, 4, 128], f32)
    for ib in range(4):
        ps_t2 = psC.tile([32, 128], f32, tag="tr2", bufs=1)
        nc.tensor.transpose(ps_t2, h_out_sb[:, ib, :], ident128)
        nc.scalar.copy(out=houtT[:, ib, :], in_=ps_t2)
    ps_s2 = psC.tile([2, R], f32)
    nc.tensor.matmul(ps_s2, lhsT=ao_sb, rhs=houtT.rearrange("p a b -> p (a b)"),
                     start=True, stop=True)
    s2_sb = sc.tile([2, R], f32)
    nc.vector.tensor_copy(out=s2_sb, in_=ps_s2)
    nc.sync.dma_start(out=cc_s2_in, in_=s2_sb)
    h_out_bf = sc.tile([128, 4, NCLASS], bf16)
    nc.vector.tensor_copy(out=h_out_bf, in_=h_out_sb)
    for ib in range(4):
        nc.sync.dma_start(out=cc_ho_in[ib * 128:(ib + 1) * 128, :],
                          in_=h_out_bf[:, ib, :])

    nc.gpsimd.collective_compute(AG, mybir.AluOpType.bypass, replica_groups=groups,
                                 ins=[cc_s2_in[:]], outs=[cc_s2_out[:]])
    nc.gpsimd.collective_compute(AG, mybir.AluOpType.bypass, replica_groups=groups,
                                 ins=[cc_ho_in[:]], outs=[cc_ho_out[:]])

    # src2 (local rows) via DRAM-broadcast round trip
    row2 = cc_s2_in[0:1, :]
    bc2 = bass.AP(tensor=row2.tensor, offset=row2.offset, ap=[[0, 128]] + row2.ap[1:])
    src2f = sc.tile([128, R], f32)
    nc.sync.dma_start(out=src2f, in_=bc2)
    nc.vector.tensor_copy(out=src2_bc, in_=src2f)
    for core in range(NCORES):
        nc.sync.dma_start(
            out=s2dst[:, core * 4:(core + 1) * 4],
            in_=cc_s2_out[core, 1, :].rearrange("(l p) -> p l", p=128))
    nc.sync.dma_start(
        out=h2_rhs[:, :, 0:NCLASS].rearrange("p (n l) o -> p n l o", n=NCORES),
        in_=cc_ho_out.rearrange("n (l p) o -> p n l o", p=128))

    psC_ctx.close()

    # =================== Stage D: layer-2 attention + log_softmax ==========
    stD = ExitStack()
    sd = stD.enter_context(tc.tile_pool(name="sd", bufs=1))
    psD = stD.enter_context(tc.tile_pool(name="psD", bufs=1, space="PSUM"))

    ps_o2T = psD.tile([NCLASS + 1, R], f32)

    def sink2(cnt, jb, qs):
        nc.tensor.matmul(ps_o2T, lhsT=h2_rhs[:, jb, :], rhs=qs,
                         start=(cnt == 0), stop=(cnt == NJB - 1))

    pat2 = PATTERNS[0]
    cnt2 = 0
    items2 = [('A', g) for g in range(8) if pat2[g] == 'A']
    l2_gs = [g for g in range(8) if pat2[g] == 'L']
    while l2_gs:
        if len(l2_gs) >= 2:
            items2.append(('L', (l2_gs[0], l2_gs[1]))); l2_gs = l2_gs[2:]
        else:
            items2.append(('L', l2_gs[0])); l2_gs = l2_gs[1:]
    for route, g in items2:
        cnt2 = emit_group(psD, route, g, src2_bc,
                          lambda jb: s2dst[:, jb:jb + 1], sink2, cnt2)

    o2T_sb = sd.tile([NCLASS + 1, R], f32)
    nc.scalar.copy(out=o2T_sb, in_=ps_o2T)
    for ib in range(4):
        ps_row = psD.tile([128, NCLASS + 1], f32, tag="o2row", bufs=2)
        nc.tensor.transpose(ps_row, o2T_sb[:, ib * 128:(ib + 1) * 128], ident33)
        dinv2 = sd.tile([128, 1], f32, tag="dinv2", bufs=2)
        nc.vector.reciprocal(out=dinv2, in_=ps_row[:, NCLASS:NCLASS + 1])
        o2 = sd.tile([128, NCLASS], f32, tag="o2", bufs=2)
        nc.vector.tensor_scalar(out=o2, in0=ps_row[:, 0:NCLASS], scalar1=dinv2,
                                scalar2=None, op0=OP.mult)
        mx = sd.tile([128, 1], f32, tag="mx", bufs=2)
        nc.vector.tensor_reduce(out=mx, in_=o2, axis=mybir.AxisListType.X, op=OP.max)
        negmx = sd.tile([128, 1], f32, tag="negmx", bufs=2)
        nc.vector.tensor_scalar(out=negmx, in0=mx, scalar1=-1.0, scalar2=None,
                                op0=OP.mult)
        eo = sd.tile([128, NCLASS], f32, tag="eo", bufs=2)
        nc.scalar.activation(out=eo, in_=o2, func=AF.Exp, bias=negmx)
        se = sd.tile([128, 1], f32, tag="se", bufs=2)
        nc.vector.tensor_reduce(out=se, in_=eo, axis=mybir.AxisListType.X, op=OP.add)
        lse = sd.tile([128, 1], f32, tag="lse", bufs=2)
        nc.scalar.activation(out=lse, in_=se, func=AF.Ln)
        b2 = sd.tile([128, 1], f32, tag="b2", bufs=2)
        nc.vector.tensor_tensor(out=b2, in0=mx, in1=lse, op=OP.add)
        res = sd.tile([128, NCLASS], f32, tag="res", bufs=2)
        nc.vector.tensor_scalar(out=res, in0=o2, scalar1=b2, scalar2=None,
                                op0=OP.subtract)
        nc.sync.dma_start(out=out[ib * 128:(ib + 1) * 128, :], in_=res)

    stD.close()
    stC.close()
    stB.close()
    cst_ctx.close()


def _prep_inputs(x, adj, W_heads, b_heads, a_heads, W_out, b_out, a_out):
    """Host-side layout prep (slicing/transpose/dtype + additive mask bias)."""
    x = np.asarray(x, dtype=np.float32)
    adj = np.asarray(adj)
    W_heads = np.asarray(W_heads, dtype=np.float32)
    b_heads = np.asarray(b_heads, dtype=np.float32)
    a_heads = np.asarray(a_heads, dtype=np.float32)
    W_out = np.asarray(W_out, dtype=np.float32)
    b_out = np.asarray(b_out, dtype=np.float32)
    a_out = np.asarray(a_out, dtype=np.float32)

    wh = np.concatenate([W_heads, b_heads[:, None, :]], axis=1).astype(BF)
    aT = np.stack([a_heads[:, :NHID], a_heads[:, NHID:]], axis=2)  # [8, 64, 2]
    aT = np.ascontiguousarray(aT).astype(BF)
    wo = np.concatenate([W_out, b_out[None, :]], axis=0).astype(BF)  # [513, 32]
    ao = np.stack([a_out[:NCLASS], a_out[NCLASS:]], axis=1)  # [32, 2]
    ao = np.ascontiguousarray(ao)

    in_maps = []
    for c in range(NCORES):
        rs = slice(c * R, (c + 1) * R)
        xTc = np.concatenate([np.ascontiguousarray(x[rs].T),
                              np.ones((1, R), np.float32)], axis=0).astype(BF)
        mbc = ((adj[rs].T.astype(np.float32) - 1.0) * MASK_BIG).astype(BF)
        mbc = np.ascontiguousarray(mbc)
        in_maps.append({"xT": xTc, "wh": wh, "mb": mbc, "aT": aT,
                        "wo": wo, "ao": ao})
    return in_maps


def kernel(**inputs) -> np.ndarray:
    if "nc" not in _cached:
        _cached["nc"] = _build_program()
    nc = _cached["nc"]
    in_maps = _prep_inputs(**inputs)
    last_err = None
    for _attempt in range(3):
        try:
            res = run_bass_kernel_spmd(nc, in_maps, list(range(NCORES)))
            return np.concatenate([res.results[c]["out"] for c in range(NCORES)],
                                  axis=0)
        except Exception as e:  # transient device errors: retry
            last_err = e
            time.sleep(2)
    raise last_err
